# revision 29
# baseline (speedup 1.0000x reference)
"""LCNNConv2d (dictionary 1x1 conv + sparse lookup combine) on 8 TRN2 NeuronCores.

Math: out[b,o,h,w] = sum_d w2[o,d] * sum_c dict[d,c] * x[b,c,h,w]
                   = sum_c (w2 @ dict)[o,c] * x[b,c,h,w]
with w2 the [O,D] scatter of lookup_coefficients at lookup_indices.

The [O=256, C=64] effective weight is tiny, so it is folded on the host; the
device kernel is a memory-bound streaming matmul, data-parallel over batch:
core i handles x[2i:2i+2].

All HBM traffic is bf16 (x and out converted on host): the DMA engines are a
single shared ~360 GB/s resource, so wire bytes are the whole roofline. Per
core that is 4.2MB in + 16.8MB out = 21MB (vs 42MB in f32). The rel-err cost
of the bf16 round-trip is ~3e-3, far inside the 2e-2 gate.

Per-core layout trick: the shard [2, 64, 16384] is viewed as [128, 16384]
(partition p = 64*b + c), so every DMA moves full-128-partition tiles. Two
zero-padded stationary weights (rows 0:64 <- W_eff.T for batch 0; rows 64:128
for batch 1) select the right batch during the 128-deep contraction.

DMA plumbing: input loads go through SWDGE (gpsimd) while output stores
alternate between the two HWDGE rings (scalar / sync); weight loads ride the
scalar ring so the first x load is never queued behind them.
"""

import numpy as np

B, C_IN, H, W = 16, 64, 128, 128
C_OUT, D_SIZE, SPARSITY = 256, 512, 4
N_CORES = 8
BPC = B // N_CORES           # batches per core = 2
HW = H * W                   # 16384
G = 1024                     # hw columns per tile (256KB bf16 DMAs)
PSW = 1024                   # psum tile width (2 banks; one copy per out tile)

_cached = {}


def _build_program(
    G=G,
    xbufs=16,
    obufs=16,
    psbufs=4,
    psum_w=PSW,
    n_hwdge_in=7,
    in_rings=None,
    copy_engines=None,
    bm_order="bm",
    ring0=1,
):
    """Build (once per config) the per-core Bass program: out = W @ xs.

    n_hwdge_in: how many leading x-tile loads ride the sync HWDGE ring
    (fastest issue path, idle early) before falling back to SWDGE; -1 means
    all inputs on sync (no SWDGE at all). in_rings (optional): explicit
    per-g ring pattern, a string of 's'(sync)/'g'(gpsimd)/'a'(scalar)
    overriding n_hwdge_in.
    """
    key = (
        G, xbufs, obufs, psbufs, psum_w, n_hwdge_in, in_rings, copy_engines,
        bm_order, ring0,
    )
    if key in _cached:
        return _cached[key]

    import concourse.bass as bass  # noqa: F401
    import concourse.tile as tile
    from concourse import bacc, mybir

    f32 = mybir.dt.float32
    bf16 = mybir.dt.bfloat16
    nc = bacc.Bacc("TRN2", target_bir_lowering=False, debug=False)

    xs = nc.dram_tensor("xs", [2 * C_IN, HW], bf16, kind="ExternalInput").ap()
    # W_eff.T duplicated on both partition halves: batch b's matmuls read
    # the [64b:64b+64] slice of both operands (row-aligned K=64 contraction)
    w = nc.dram_tensor("w", [2 * C_IN, C_OUT], bf16, kind="ExternalInput").ap()
    # out[b, m, o, hw] with o-chunk m of 128: host reshapes to [2, 256, HW]
    out = nc.dram_tensor(
        "out", [BPC, C_OUT // 128, 128, HW], bf16, kind="ExternalOutput"
    ).ap()

    with tile.TileContext(nc) as tc:
        with (
            tc.tile_pool(name="w", bufs=1) as wpool,
            tc.tile_pool(name="xin", bufs=xbufs) as xpool,
            tc.tile_pool(name="ostage", bufs=obufs) as opool,
            tc.tile_pool(name="ps", bufs=psbufs, space="PSUM") as pspool,
        ):
            wt = wpool.tile([128, C_OUT], bf16)
            nc.scalar.dma_start(wt, w)

            out_rings = [nc.scalar, nc.sync]
            ring_by_ch = {
                "s": nc.sync,
                "g": nc.gpsimd,
                "a": nc.scalar,
                "v": nc.vector,
            }
            di = 0
            for g in range(HW // G):
                xt = xpool.tile([128, G], bf16)
                if in_rings is not None:
                    in_ring = ring_by_ch[in_rings[g]]
                else:
                    in_ring = (
                        nc.sync if (n_hwdge_in < 0 or g < n_hwdge_in) else nc.gpsimd
                    )
                in_ring.dma_start(xt, xs[:, g * G : (g + 1) * G])
                bm = [(b, m) for b in range(BPC) for m in range(C_OUT // 128)]
                if bm_order == "mb":
                    bm = [(b, m) for m in range(C_OUT // 128) for b in range(BPC)]
                for b, m in bm:
                    if True:
                        ot = opool.tile([128, G], bf16, tag="ot")
                        for s0 in range(max(G // psum_w, 1)):
                            pw = min(psum_w, G)
                            ps = pspool.tile([128, pw], f32)
                            for s1 in range(pw // 512):
                                col = s0 * pw + s1 * 512
                                # bf16 matmul: full PE rate, f32 accumulate
                                nc.tensor.matmul(
                                    ps[:, s1 * 512 : (s1 + 1) * 512],
                                    wt[b * C_IN : (b + 1) * C_IN, m * 128 : (m + 1) * 128],
                                    xt[b * C_IN : (b + 1) * C_IN, col : col + 512],
                                    start=True,
                                    stop=True,
                                )
                            # cast f32 psum -> bf16 staging during the drain
                            dst = ot[:, s0 * pw : (s0 + 1) * pw]
                            if copy_engines is None:
                                nc.any.tensor_copy(dst, ps)
                            else:
                                ch = copy_engines[di % len(copy_engines)]
                                if ch == "a":
                                    nc.scalar.copy(dst, ps)
                                else:
                                    ring_by_ch[ch].tensor_copy(dst, ps)
                        out_rings[(di + ring0) % 2].dma_start(
                            out[b, m, :, g * G : (g + 1) * G], ot
                        )
                        di += 1

    nc.compile()
    _cached[key] = nc
    return nc


def _bf16(a):
    import ml_dtypes

    return np.asarray(a, np.float32).astype(ml_dtypes.bfloat16)


def _effective_weights(dictionary, lookup_coefficients, lookup_indices):
    """Fold conv dictionary + sparse combine into two padded lhsT weights."""
    idx = np.asarray(lookup_indices).reshape(C_OUT, -1).astype(np.int64)
    coeff = np.asarray(lookup_coefficients, np.float32).reshape(C_OUT, -1)
    w2 = np.zeros((C_OUT, D_SIZE), np.float32)
    np.add.at(w2, (np.arange(C_OUT)[:, None], idx), coeff)
    w_eff = w2 @ np.asarray(dictionary, np.float32).reshape(D_SIZE, C_IN)  # [O, C]
    wT = np.ascontiguousarray(np.concatenate([w_eff.T, w_eff.T], axis=0))
    return _bf16(wT), w_eff


def make_in_maps(x, dictionary, lookup_coefficients, lookup_indices):
    w_bf, w_eff = _effective_weights(
        dictionary, lookup_coefficients, lookup_indices
    )
    xf = _bf16(np.asarray(x, np.float32).reshape(B, C_IN, HW))
    maps = [
        {
            "xs": np.ascontiguousarray(
                xf[i * BPC : (i + 1) * BPC].reshape(BPC * C_IN, HW)
            ),
            "w": w_bf,
        }
        for i in range(N_CORES)
    ]
    return maps, w_eff, xf


def _spot_check(out, w_eff, xf, rng):
    """Verify a random sample of outputs on the host (guards a rare
    first-execution flake seen on the PJRT path). Loose tol: the device
    math is bf16 in/out, the host reference here is f32-from-bf16-x."""
    n = 2048
    bs = rng.integers(0, B, n)
    os_ = rng.integers(0, C_OUT, n)
    ps = rng.integers(0, HW, n)
    ref = np.einsum(
        "nc,nc->n", w_eff[os_], xf[bs, :, ps].astype(np.float32)
    )
    got = out.reshape(B, C_OUT, HW)[bs, os_, ps]
    tol = 3e-2 * max(np.abs(ref).max(), 1.0)
    return np.all(np.isfinite(got)) and np.abs(got - ref).max() < tol


def kernel(x, dictionary, lookup_coefficients, lookup_indices):
    from concourse.bass_utils import run_bass_kernel_spmd

    nc = _build_program()
    in_maps, w_eff, xf = make_in_maps(
        x, dictionary, lookup_coefficients, lookup_indices
    )
    rng = np.random.default_rng(0)
    for _attempt in range(3):
        res = run_bass_kernel_spmd(nc, in_maps, core_ids=list(range(N_CORES)))
        out = np.concatenate(
            [
                res.results[i]["out"]
                .astype(np.float32)
                .reshape(BPC, C_OUT, H, W)
                for i in range(N_CORES)
            ],
            axis=0,
        )
        if _spot_check(out, w_eff, xf, rng):
            break
    return out
